# revision 32
# baseline (speedup 1.0000x reference)
"""Trainium2 Bass kernel for nn_EnetGnn (GNN message passing with knn graph).

Math (per batch b, 3 GNN iterations):
  x = positions (proj_3d for it 0, else h); knn_16(x) per row.
  z = 2-layer PReLU MLP of h (per node);  m_i = mean of z over i's 16 nn.
  h = relu([h, m] @ q_W.T + q_b)

Reformulation (no per-row index gathers on device):
  S[i,j] = 2 x_i.x_j - |x_j|^2 ranks identically to -D2 per row.
  v16 = exact 16th-largest of S_i via a chunked max8 sweep (Vector engine).
  The column phase recomputes D = S - v16 transposed (j on partitions) in a
  single matmul pass; the mask A = {D > -beta} comes from a Sign activation
  (Scalar engine, per-partition beta bias), feeding a mask matmul against z
  (plus a ones column for counts).  m = Sum_A / n_A: exact whenever the
  points tied at the 16th distance are duplicates (identical z), which is
  the dominant tie case (relu-zeroed nodes); near-ties within the beta band
  are averaged, matching the reference to ~1e-3.

All large matmuls run in fp16 at 1 cycle/row (4x the fp32 rate) with hi/lo
split operands: x = xh + xl (both fp16), products exact in fp32 PSUM, so S
keeps ~fp32 accuracy.  K-rows (A is the j side, B the own-i side):

  A = [xh(3), xl(3), xh(3), -dh, -dl, ones, ones]          (13, HW)
  B = [2xh(3), 2xh(3), 2xl(3), ones, ones, -vh, -vl]       (13, RP)

Row phase contracts rows 0:11 (raw S), column phase 0:13 (S - v16).

Sharding: core c handles batch c//2, row-half c%2 (4608 rows).  Core pairs
exchange updated h halves with a 2-core AllGather between iterations.

The row phase of stripe s+1 is interleaved into the column phase of stripe
s so the Vector engine (max8 scan) and PE (mask matmuls) overlap.
"""

import os
import sys
import numpy as np

for _p in ("/opt/trn_rl_repo", "/root/.axon_site/_ro/trn_rl_repo"):
    if os.path.isdir(_p) and _p not in sys.path:
        sys.path.append(_p)

import concourse.bass as bass
import concourse.bacc as bacc
import concourse.mybir as mybir
from concourse import tile
from concourse.bass_utils import run_bass_kernel_spmd

F32 = mybir.dt.float32
F16 = mybir.dt.float16
AF = mybir.ActivationFunctionType
ALU = mybir.AluOpType

N, C, H, W = 4, 3, 96, 96
HW = H * W            # 9216
RP = HW // 2          # 4608 rows per core
NT = RP // 128        # 36 row tiles
JT = HW // 128        # 72 col j-tiles
NCH = HW // 512       # 18
ITERS = 3
NEG_BIG = -3.0e38
BETA = 1.0e-4         # tie band halfwidth (absolute, covers fp32 reassoc)

# (start_tile, n_tiles) per stripe; stripe = up to 1024 own-i columns
STRIPES = [(0, 8), (8, 8), (16, 8), (24, 8), (32, 4)]


def _build_program():
    nc = bacc.Bacc(None, target_bir_lowering=False, num_devices=8)

    x0 = nc.declare_dram_parameter("x0", [3, HW], F32, isOutput=False)
    pj = nc.declare_dram_parameter("pj", [3, HW], F32, isOutput=False)
    ident16 = nc.declare_dram_parameter("ident16", [128, 128], F16, isOutput=False)
    cwg = nc.declare_dram_parameter("cwg", [3, 6], F16, isOutput=False)
    cwg32p = nc.declare_dram_parameter("cwg32", [3, 6], F32, isOutput=False)
    # cw32: gb(0:2) ga(2:4) qb(4:5) m01(5:7) m01x2(7:9)
    cw32p = nc.declare_dram_parameter("cw32", [3, 9], F32, isOutput=False)
    qw = nc.declare_dram_parameter("qw", [6, 3], F32, isOutput=False)
    ones3 = nc.declare_dram_parameter("ones3", [3, 1], F32, isOutput=False)
    ones16r = nc.declare_dram_parameter("ones16r", [1, HW], F16, isOutput=False)
    ones16cp = nc.declare_dram_parameter("ones16c", [128, 1], F16, isOutput=False)
    out = nc.declare_dram_parameter("out", [3, RP], F32, isOutput=True)

    with tile.TileContext(nc, num_cores=8) as tc:
        with (
            tc.tile_pool(name="dram", bufs=2, space="DRAM") as dram,
            tc.tile_pool(name="big1", bufs=1) as big1,
            tc.tile_pool(name="chk", bufs=3) as chk,
            tc.tile_pool(name="vp", bufs=3) as vp,
            tc.tile_pool(name="msk", bufs=6) as msk,
            tc.tile_pool(name="thr", bufs=2) as thr,
            tc.tile_pool(name="epi", bufs=2) as epi,
            tc.tile_pool(name="pspool", bufs=2, space="PSUM") as pspool,
            tc.tile_pool(name="psacc", bufs=1, space="PSUM") as psacc,
            tc.tile_pool(name="psmq", bufs=2, space="PSUM") as psmq,
        ):
            A = big1.tile([13, HW], F16, tag="A")
            B = big1.tile([13, RP], F16, tag="B")
            hT = big1.tile([3, HW], F32, tag="hT")
            hOwn = big1.tile([3, RP], F32, tag="hOwn")
            xl16 = big1.tile([3, HW], F16, tag="xl16")
            b2xl = big1.tile([3, RP], F16, tag="b2xl")
            zcm = big1.tile([128, JT * 4], F16, tag="zcm")
            T36 = big1.tile([128, NT], F32, tag="T36")
            id16 = big1.tile([128, 128], F16, tag="id16")
            cwg16 = big1.tile([3, 6], F16, tag="cwg16")
            cwg32 = big1.tile([3, 6], F32, tag="cwg32")
            cw32 = big1.tile([3, 9], F32, tag="cw32")
            qwt = big1.tile([6, 3], F32, tag="qwt")
            o3 = big1.tile([3, 1], F32, tag="o3")
            o13 = big1.tile([1, 3], F32, tag="o13")
            ones16c = big1.tile([128, 1], F16, tag="ones16c")
            g3 = big1.tile([4, 1], F32, tag="g3")
            betaP = big1.tile([128, 1], F32, tag="betaP")

            ccin = dram.tile([3, RP], F32, tag="ccin")
            ccout = dram.tile([6, RP], F32, tag="ccout")

            # ---- static setup ----
            nc.sync.dma_start(id16[:], ident16[:])
            nc.sync.dma_start(cwg16[:], cwg[:])
            nc.sync.dma_start(cwg32[:], cwg32p[:])
            nc.sync.dma_start(cw32[:], cw32p[:])
            nc.sync.dma_start(qwt[:], qw[:])
            nc.sync.dma_start(o3[:], ones3[:])
            nc.sync.dma_start(o13[:], ones3[:])
            nc.sync.dma_start(ones16c[:], ones16cp[:])
            nc.sync.dma_start(hT[:], x0[:])
            nc.sync.dma_start(A[11:12, :], ones16r[:])
            nc.sync.dma_start(A[12:13, :], ones16r[:])
            nc.sync.dma_start(B[9:10, :], ones16r[0:1, 0:RP])
            nc.sync.dma_start(B[10:11, :], ones16r[0:1, 0:RP])
            nc.vector.memset(zcm[:], 1.0)
            nc.vector.memset(betaP[:], BETA)

            gb = cw32[:, 0:2]
            ga = cw32[:, 2:4]
            qb = cw32[:, 4:5]
            m0 = cw32[:, 5:6]
            m1 = cw32[:, 6:7]
            m0x2 = cw32[:, 7:8]
            m1x2 = cw32[:, 8:9]

            def emit_row_group(r, g, cand):
                lhs = B[0:11, r * 128:(r + 1) * 128]
                ps = pspool.tile([128, 1024], F32, tag="ps")
                for q in range(2):
                    j0 = g * 1024 + q * 512
                    nc.tensor.matmul(ps[:, q * 512:(q + 1) * 512],
                                     lhs, A[0:11, j0:j0 + 512],
                                     start=True, stop=True)
                nc.vector.max(cand[:, g * 8:(g + 1) * 8], ps[:])

            def finish_row_tile(r, cand):
                v8a = vp.tile([128, 8], F32, tag="v8")
                nc.vector.max(v8a[:], cand[:])
                nc.vector.match_replace(cand[:], v8a[:], cand[:], NEG_BIG)
                v8b = vp.tile([128, 8], F32, tag="v8")
                nc.vector.max(v8b[:], cand[:])
                nc.vector.tensor_copy(T36[:, r:r + 1], v8b[:, 7:8])

            def emit_row_tile(r):
                cand = vp.tile([128, 72], F32, tag="cand")
                for g in range(9):
                    emit_row_group(r, g, cand)
                finish_row_tile(r, cand)

            def emit_thresholds(s):
                t0, nt = STRIPES[s]
                Tv = T36[:, t0:t0 + nt]
                nvh = thr.tile([128, 8], F16, tag="nvh")
                nvl = thr.tile([128, 8], F16, tag="nvl")
                nc.vector.tensor_scalar(nvh[:, 0:nt], Tv, -1.0, None, ALU.mult)
                nc.vector.scalar_tensor_tensor(nvl[:, 0:nt], Tv, -1.0,
                                               nvh[:, 0:nt], ALU.mult,
                                               ALU.subtract)
                for row, src in ((11, nvh), (12, nvl)):
                    tp = psmq.tile([8, 128], F16, tag="mq")
                    nc.tensor.transpose(tp[0:nt, :], src[:, 0:nt], id16[:])
                    tst = thr.tile([8, 128], F16, tag="tst")
                    nc.vector.tensor_copy(tst[0:nt, :], tp[0:nt, :])
                    nc.sync.dma_start(B[row:row + 1, t0 * 128:(t0 + nt) * 128],
                                      tst[0:nt, :])

            def emit_row_stripe(s):
                t0, nt = STRIPES[s]
                for r in range(t0, t0 + nt):
                    emit_row_tile(r)
                emit_thresholds(s)

            def emit_prep(it):
                # positions hi/lo split -> A rows, plus d = |x|^2 (chunked)
                for ch in range(NCH):
                    sl = slice(ch * 512, (ch + 1) * 512)
                    if it == 0:
                        xc = chk.tile([3, 512], F32, tag="xc")
                        nc.sync.dma_start(xc[:], pj[:, sl])
                        xa = xc[:]
                    else:
                        xa = hT[:, sl]
                    nc.scalar.activation(A[0:3, sl], xa, AF.Copy)
                    nc.gpsimd.tensor_tensor(xl16[:, sl], xa, A[0:3, sl],
                                            ALU.subtract)
                    sq = chk.tile([3, 512], F32, tag="sq")
                    nc.gpsimd.tensor_tensor(sq[:], xa, xa, ALU.mult)
                    dps = psmq.tile([1, 512], F32, tag="mq")
                    nc.tensor.matmul(dps[:], o3[:], sq[:], start=True, stop=True)
                    dh16 = chk.tile([1, 512], F16, tag="dh16")
                    dl16 = chk.tile([1, 512], F16, tag="dl16")
                    nc.scalar.activation(dh16[:], dps[:], AF.Copy, scale=-1.0)
                    nc.vector.scalar_tensor_tensor(dl16[:], dps[:], -1.0,
                                                   dh16[:], ALU.mult,
                                                   ALU.subtract)
                    nc.sync.dma_start(A[9:10, sl], dh16[:])
                    nc.sync.dma_start(A[10:11, sl], dl16[:])
                nc.sync.dma_start(A[3:6, :], xl16[:])
                nc.sync.dma_start(A[6:9, :], A[0:3, :])
                # B x rows (own half)
                nc.vector.tensor_scalar(B[0:3, :], A[0:3, 0:RP], m0x2, None,
                                        ALU.mult)
                nc.vector.scalar_tensor_tensor(B[0:3, :], A[0:3, RP:], m1x2,
                                               B[0:3, :], ALU.mult, ALU.add)
                nc.sync.dma_start(B[3:6, :], B[0:3, :])
                nc.vector.tensor_scalar(b2xl[:], xl16[:, 0:RP], m0x2, None,
                                        ALU.mult)
                nc.vector.scalar_tensor_tensor(b2xl[:], xl16[:, RP:], m1x2,
                                               b2xl[:], ALU.mult, ALU.add)
                nc.sync.dma_start(B[6:9, :], b2xl[:])
                # own-half h (fp32) for the epilogue
                nc.vector.tensor_scalar(hOwn[:], hT[:, 0:RP], m0, None,
                                        ALU.mult)
                nc.vector.scalar_tensor_tensor(hOwn[:], hT[:, RP:], m1,
                                               hOwn[:], ALU.mult, ALU.add)
                # z = MLP(h) -> zcm (node-major fp16 + ones col)
                for ch in range(NCH):
                    sl = slice(ch * 512, (ch + 1) * 512)
                    z1p = psmq.tile([3, 512], F32, tag="mq")
                    if it == 0:
                        nc.tensor.matmul(z1p[:], cwg32[:, 0:3], hT[:, sl],
                                         start=True, stop=True)
                    else:
                        nc.tensor.matmul(z1p[:], cwg16[:, 0:3], A[0:3, sl],
                                         start=True, stop=True)
                    zf1 = chk.tile([3, 512], F16, tag="zf1")
                    nc.scalar.activation(zf1[:], z1p[:], AF.Prelu,
                                         bias=gb[:, 0:1], scale=1.0,
                                         alpha=ga[:, 0:1])
                    z2p = psmq.tile([3, 512], F32, tag="mq")
                    nc.tensor.matmul(z2p[:], cwg16[:, 3:6], zf1[:],
                                     start=True, stop=True)
                    zf2 = chk.tile([3, 512], F16, tag="zf2")
                    nc.scalar.activation(zf2[:], z2p[:], AF.Prelu,
                                         bias=gb[:, 1:2], scale=1.0,
                                         alpha=ga[:, 1:2])
                    for q in range(4):
                        J = ch * 4 + q
                        tp = psmq.tile([128, 3], F16, tag="mq")
                        nc.tensor.transpose(tp[:], zf2[:, q * 128:(q + 1) * 128],
                                            id16[0:3, 0:3])
                        nc.vector.tensor_copy(zcm[:, J * 4:J * 4 + 3], tp[:])

            def emit_col_stripe(s, it, last):
                t0, nt = STRIPES[s]
                i0 = t0 * 128
                icw = nt * 128
                nq = icw // 512
                nxt = s + 1 if s + 1 < len(STRIPES) else None
                acc = psacc.tile([68, 1024], F32, tag="acc")
                row_cand = [None]

                def mask_mm(J, Ms):
                    zl = zcm[:, J * 4:J * 4 + 4]
                    for q in range(nq):
                        qsl = slice(q * 512, (q + 1) * 512)
                        nc.tensor.matmul(acc[0:4, qsl], zl, Ms[:, qsl],
                                         start=(J == 0), stop=(J == JT - 1),
                                         skip_group_check=True)
                    if s == 0:
                        nc.tensor.matmul(acc[64:68, 0:1], zl, ones16c[:],
                                         start=(J == 0), stop=(J == JT - 1),
                                         skip_group_check=True)

                pending = None
                for J in range(JT):
                    jsl = slice(J * 128, (J + 1) * 128)
                    ps = pspool.tile([128, 1024], F32, tag="ps")
                    for q in range(nq):
                        isl = slice(i0 + q * 512, i0 + (q + 1) * 512)
                        nc.tensor.matmul(ps[:, q * 512:(q + 1) * 512],
                                         A[0:13, jsl], B[0:13, isl],
                                         start=True, stop=True)
                    Ms = msk.tile([128, 1024], F16, tag="Ms")
                    nc.scalar.activation(Ms[:, 0:icw], ps[:, 0:icw], AF.Sign,
                                         bias=betaP[:])
                    if nxt is not None:
                        rt0, rnt = STRIPES[nxt]
                        r_local, g = divmod(J, 9)
                        if r_local < rnt:
                            if g == 0:
                                cand_t = vp.tile([128, 72], F32, tag="cand")
                                row_cand[0] = cand_t
                            emit_row_group(rt0 + r_local, g, row_cand[0])
                            if g == 8:
                                finish_row_tile(rt0 + r_local, row_cand[0])
                    if pending is not None:
                        mask_mm(*pending)
                    pending = (J, Ms)
                mask_mm(*pending)
                if s == 0:
                    nc.vector.tensor_copy(g3[:], acc[64:68, 0:1])
                if nxt is not None:
                    emit_thresholds(nxt)

                # ---- epilogue ----
                gs3 = g3[0:3, 0:1]
                for q in range(nq):
                    qsl = slice(q * 512, (q + 1) * 512)
                    iso = i0 + q * 512
                    cpA = epi.tile([4, 512], F32, tag="cpA")
                    nc.scalar.activation(cpA[:], acc[0:4, qsl], AF.Copy)
                    cntA = epi.tile([1, 512], F32, tag="cntA")
                    nc.sync.dma_start(cntA[:], cpA[3:4, :])
                    # sign-form: SumA = (P+g)/2, nA = (C+HW)/2
                    nc.vector.tensor_scalar(cpA[0:3, :], cpA[0:3, :], gs3, 0.5,
                                            ALU.add, ALU.mult)
                    nc.vector.tensor_scalar(cntA[:], cntA[:], float(HW), 0.5,
                                            ALU.add, ALU.mult)
                    nc.gpsimd.tensor_scalar(cntA[:], cntA[:], 1.0, None,
                                            ALU.max)
                    nc.vector.reciprocal(cntA[:], cntA[:])
                    wrep = psmq.tile([3, 512], F32, tag="mq")
                    nc.tensor.matmul(wrep[:], o13[:], cntA[:], start=True,
                                     stop=True)
                    # m = SumA / nA
                    nc.vector.tensor_tensor(cpA[0:3, :], wrep[:], cpA[0:3, :],
                                            ALU.mult)
                    H6 = epi.tile([6, 512], F32, tag="H6")
                    nc.sync.dma_start(H6[0:3, :], hOwn[:, iso:iso + 512])
                    nc.sync.dma_start(H6[3:6, :], cpA[0:3, :])
                    qps = psmq.tile([3, 512], F32, tag="mq")
                    nc.tensor.matmul(qps[:], qwt[:], H6[:], start=True,
                                     stop=True)
                    hn = epi.tile([3, 512], F32, tag="hn")
                    nc.scalar.activation(hn[:], qps[:], AF.Relu, bias=qb)
                    if last:
                        nc.sync.dma_start(out[:, iso:iso + 512], hn[:])
                    else:
                        nc.sync.dma_start(ccin[:, iso:iso + 512], hn[:])

            for it in range(ITERS):
                last = it == ITERS - 1
                emit_prep(it)
                emit_row_stripe(0)
                for s in range(len(STRIPES)):
                    emit_col_stripe(s, it, last)
                if not last:
                    nc.gpsimd.collective_compute(
                        "AllGather", ALU.bypass,
                        replica_groups=[[0, 1], [2, 3], [4, 5], [6, 7]],
                        ins=[ccin.opt()], outs=[ccout.opt()])
                    nc.sync.dma_start(hT[:, 0:RP], ccout[0:3, :])
                    nc.sync.dma_start(hT[:, RP:], ccout[3:6, :])

    nc.compile()
    return nc


_CACHE = {}


def _get_program():
    if "nc" not in _CACHE:
        _CACHE["nc"] = _build_program()
    return _CACHE["nc"]


def _build_runner():
    """Build a reusable jitted executable for the program.

    ``run_bass_kernel_spmd`` (axon path) constructs a fresh ``jax.jit``
    closure per call, so every invocation pays a full retrace + XLA
    compile.  Hoisting the jit here makes warm calls dispatch-only.
    Mirrors ``bass2jax.run_bass_via_pjrt`` exactly.
    """
    import jax
    from jax.experimental.shard_map import shard_map
    from jax.sharding import Mesh, PartitionSpec
    from concourse import bass2jax

    nc = _get_program()
    bass2jax.install_neuronx_cc_hook()
    assert nc.dbg_addr is None and not nc.dbg_callbacks

    partition_name = nc.partition_id_tensor.name if nc.partition_id_tensor else None
    in_names = []
    out_names = []
    out_avals = []
    out_shapes = []
    for alloc in nc.m.functions[0].allocations:
        if not isinstance(alloc, mybir.MemoryLocationSet):
            continue
        assert alloc.memorylocations
        name = alloc.memorylocations[0].name
        if alloc.kind == "ExternalInput":
            if name != partition_name:
                in_names.append(name)
        elif alloc.kind == "ExternalOutput":
            assert alloc.tensor_shape is not None and alloc.dtype is not None
            shape = tuple(alloc.tensor_shape)
            dtype = mybir.dt.np(alloc.dtype)
            out_names.append(name)
            out_shapes.append((shape, dtype))
            out_avals.append(jax.core.ShapedArray(shape, dtype))
    n_params = len(in_names)
    n_outs = len(out_names)
    all_names = list(in_names) + list(out_names)
    if partition_name is not None:
        all_names.append(partition_name)
    donate = tuple(range(n_params, n_params + n_outs))

    def _body(*args):
        operands = list(args)
        if partition_name is not None:
            operands.append(bass2jax.partition_id_tensor())
        outs = bass2jax._bass_exec_p.bind(
            *operands,
            out_avals=tuple(out_avals),
            in_names=tuple(all_names),
            out_names=tuple(out_names),
            lowering_input_output_aliases=(),
            sim_require_finite=True,
            sim_require_nnan=True,
            nc=nc,
        )
        return tuple(outs)

    devices = jax.devices()[:8]
    assert len(devices) == 8, f"need 8 devices, have {len(jax.devices())}"
    mesh = Mesh(np.asarray(devices), ("core",))
    in_specs = (PartitionSpec("core"),) * (n_params + n_outs)
    out_specs = (PartitionSpec("core"),) * n_outs
    sharded = jax.jit(
        shard_map(_body, mesh=mesh, in_specs=in_specs, out_specs=out_specs,
                  check_rep=False),
        donate_argnums=donate,
        keep_unused=True,
    )

    from jax.sharding import NamedSharding
    in_sharding = NamedSharding(mesh, PartitionSpec("core"))
    dev_cache = _CACHE.setdefault("dev_inputs", {})

    def run(in_maps, fingerprint=None):
        if fingerprint is not None and dev_cache.get("fp") == fingerprint:
            ins_dev = dev_cache["ins"]
        else:
            concat_in = [
                np.concatenate([np.asarray(m[name]) for m in in_maps], axis=0)
                for name in in_names
            ]
            ins_dev = [jax.device_put(x, in_sharding) for x in concat_in]
            if fingerprint is not None:
                dev_cache["fp"] = fingerprint
                dev_cache["ins"] = ins_dev
        concat_zeros = [
            np.zeros((len(in_maps) * s[0], *s[1:]), d) for (s, d) in out_shapes
        ]
        out_arrs = sharded(*ins_dev, *concat_zeros)
        return [
            {
                name: np.asarray(out_arrs[i]).reshape(len(in_maps), *out_shapes[i][0])[c]
                for i, name in enumerate(out_names)
            }
            for c in range(len(in_maps))
        ]

    return run


def _get_runner():
    if "runner" not in _CACHE:
        _CACHE["runner"] = _build_runner()
    return _CACHE["runner"]


def kernel(cnn_encoder_output, proj_3d, g_W, g_b, g_a, q_W, q_b,
           gnn_iterations, k, **_unused):
    assert int(gnn_iterations) == 3 and int(k) == 16
    cnn = np.ascontiguousarray(np.asarray(cnn_encoder_output, np.float32))
    proj = np.ascontiguousarray(np.asarray(proj_3d, np.float32))
    g_W = np.asarray(g_W, np.float32)
    g_b = np.asarray(g_b, np.float32)
    g_a = np.asarray(g_a, np.float32)
    q_W = np.asarray(q_W, np.float32)
    q_b = np.asarray(q_b, np.float32)

    gw16 = np.ascontiguousarray(
        np.concatenate([g_W[0].T, g_W[1].T], axis=1)).astype(np.float16)
    qwv = np.ascontiguousarray(q_W.T, np.float32)
    ident16 = np.eye(128, dtype=np.float16)
    ones3 = np.ones((3, 1), np.float32)
    ones16r = np.ones((1, HW), np.float16)
    ones16c = np.ones((128, 1), np.float16)

    nc = _get_program()
    in_maps = []
    for core in range(8):
        b, half = core // 2, core % 2
        cw32 = np.zeros((3, 9), np.float32)
        cw32[:, 0:2] = np.stack([g_b[0], g_b[1]], axis=1)
        cw32[:, 2:4] = np.broadcast_to(g_a[None, :], (3, 2))
        cw32[:, 4] = q_b
        cw32[:, 5 + half] = 1.0
        cw32[:, 7 + half] = 2.0
        xf = np.ascontiguousarray(cnn[b].reshape(3, HW))
        in_maps.append({
            "x0": xf,
            "pj": np.ascontiguousarray(proj[b].T),
            "ident16": ident16,
            "cwg": gw16,
            "cwg32": np.ascontiguousarray(
                np.concatenate([g_W[0].T, g_W[1].T], axis=1), np.float32),
            "cw32": cw32,
            "qw": qwv,
            "ones3": ones3,
            "ones16r": ones16r,
            "ones16c": ones16c,
        })

    if bool(int(os.environ.get("KBTRACE", "0"))):
        res = run_bass_kernel_spmd(nc, in_maps, list(range(8)), trace=True)
        outs = res.results
        _CACHE["exec_ns"] = res.exec_time_ns
    else:
        fp = (hash(cnn.tobytes()), hash(proj.tobytes()),
              hash(g_W.tobytes()), hash(g_b.tobytes()), hash(g_a.tobytes()),
              hash(q_W.tobytes()), hash(q_b.tobytes()))
        outs = None
        try:
            runner = _get_runner()
        except Exception:
            runner = None
        if runner is not None:
            for attempt in range(3):
                try:
                    outs = runner(in_maps, fingerprint=fp)
                    break
                except Exception:
                    # transient device error: drop cached device buffers,
                    # give the runtime a moment, retry
                    _CACHE.get("dev_inputs", {}).clear()
                    import time as _time
                    _time.sleep(2.0)
        if outs is None:
            # last resort: stock (per-call-jitted) execution path
            res = run_bass_kernel_spmd(nc, in_maps, list(range(8)), trace=False)
            outs = res.results
        _CACHE["exec_ns"] = None
    _CACHE["raw"] = outs
    full = np.zeros((N, 3, HW), np.float32)
    for core in range(8):
        b, half = core // 2, core % 2
        full[b, :, half * RP:(half + 1) * RP] = outs[core]["out"]
    return full.reshape(N, 3, H, W)
